# revision 2
# baseline (speedup 1.0000x reference)
"""Multi-head attention (B=2, N=2048, d_model=1024, 16 heads x 64) on 8
Trainium2 NeuronCores — bf16 matmul variant.

Sharding: batch x head-group. Core c handles batch b = c//4 and heads
4*(c%4) .. 4*(c%4)+3. Projection weights are column-sliced (rows for Wo) so
each core computes q/k/v projections only for its 4 heads, full attention
for those heads, and a partial output projection. The host sums the four
partial outputs per batch (tensor-parallel reduce on to_out) and adds bo.

All matmul operands are bf16 (validated 2.3e-3 rel err on CPU sim vs the
2e-2 gate); accumulation stays fp32 in PSUM. Inputs/weights are converted
to bf16 on the host, halving input DMA; the partial output is stored bf16
too. The softmax normalization path (reciprocal + broadcast matmul) stays
f32/f32r for accuracy.
"""

import numpy as np
import ml_dtypes

import concourse.mybir as mybir
import concourse.tile as tile
from concourse import bacc
from concourse import bass_utils
from concourse.tile_rust import add_dep_helper

F32 = mybir.dt.float32
F32R = mybir.dt.float32r
BF16 = mybir.dt.bfloat16
EXP = mybir.ActivationFunctionType.Exp

B = 2
N = 2048
D_MODEL = 1024
NHEAD = 16
DIM_HEAD = 64
SCALE = DIM_HEAD ** (-0.5)
N_CORES = 8
HEADS_PER_CORE = 4          # 2 pairs
INNER = HEADS_PER_CORE * DIM_HEAD  # 256

QB = 512                    # query block
N_QB = N // QB              # 4
N_KC = N // 128             # 16 key chunks


def build_nc():
    nc = bacc.Bacc("TRN2", target_bir_lowering=False, debug=False,
                   num_devices=N_CORES)
    # x inputs pre-tiled on host to [block, partition, chunk, col] so each
    # block DMA is one contiguous 8KB run per partition (cheap descriptors)
    xqt = nc.dram_tensor("xqt", [N_QB, 128, 8, QB], BF16,
                         kind="ExternalInput").ap()
    xkt = nc.dram_tensor("xkt", [N_QB, 128, 8, QB], BF16,
                         kind="ExternalInput").ap()
    xvt = nc.dram_tensor("xvt", [N_QB, 128, 8, QB], BF16,
                         kind="ExternalInput").ap()
    wq = nc.dram_tensor("wq", [128, 8, INNER], BF16, kind="ExternalInput").ap()
    wk = nc.dram_tensor("wk", [128, 8, INNER], BF16, kind="ExternalInput").ap()
    wv = nc.dram_tensor("wv", [128, 8, INNER], BF16, kind="ExternalInput").ap()
    wo = nc.dram_tensor("wo", [128, 2, D_MODEL], BF16,
                        kind="ExternalInput").ap()
    # bc pattern: pat4[k, p, m] = 1 where head k owns output rows m in pair p
    pat4 = nc.dram_tensor("pat4", [128, 2, 128], F32R, kind="ExternalInput").ap()
    out = nc.dram_tensor("out", [N, D_MODEL], BF16, kind="ExternalOutput").ap()

    with tile.TileContext(nc) as tc:
        with (
            tc.tile_pool(name="wpool", bufs=1) as wpool,
            tc.tile_pool(name="persist", bufs=1) as persist,
            tc.tile_pool(name="xin", bufs=3) as xin,
            tc.tile_pool(name="ering", bufs=12) as ering,
            tc.tile_pool(name="stage", bufs=3) as stage,
            tc.tile_pool(name="ps_st", bufs=2, space="PSUM") as ps_st,
            tc.tile_pool(name="ps_av", bufs=1, space="PSUM") as ps_av,
            tc.tile_pool(name="ps_misc", bufs=2, space="PSUM") as ps_misc,
        ):
            # ---- weights on sync queue, ordered by first use; halves so
            # the first projection matmuls start on the first 4 chunks ----
            wk_sb = wpool.tile([128, 8, INNER], BF16)
            nc.sync.dma_start(wk_sb[:, 0:4, :], wk[:, 0:4, :])
            nc.sync.dma_start(wk_sb[:, 4:8, :], wk[:, 4:8, :])
            wq_sb = wpool.tile([128, 8, INNER], BF16)
            nc.sync.dma_start(wq_sb[:, 0:4, :], wq[:, 0:4, :])
            nc.sync.dma_start(wq_sb[:, 4:8, :], wq[:, 4:8, :])

            qt_sb = persist.tile([128, 2, N], BF16)
            kt_sb = persist.tile([128, 2, N], BF16)
            v_sb = persist.tile([128, N_KC, HEADS_PER_CORE, DIM_HEAD + 1], BF16)
            ot_sb = persist.tile([128, 2, N], BF16)

            def emit_kt(n):
                ns = slice(n * QB, (n + 1) * QB)
                xk_t = xin.tile([128, 8, QB], BF16, tag="xin", name=f"xk_{n}")
                nc.scalar.dma_start(xk_t[:], xkt[n])
                for m in range(2):
                    pk = ps_misc.tile([128, QB], F32, tag="mp", name=f"pk{n}{m}")
                    for c in range(8):
                        nc.tensor.matmul(
                            pk[:], wk_sb[:, c, m * 128:(m + 1) * 128],
                            xk_t[:, c, :], start=(c == 0), stop=(c == 7))
                    nc.vector.tensor_copy(kt_sb[:, m, ns], pk[:])

            def emit_qt(n):
                ns = slice(n * QB, (n + 1) * QB)
                xq_t = xin.tile([128, 8, QB], BF16, tag="xin", name=f"xq_{n}")
                nc.sync.dma_start(xq_t[:], xqt[n])
                for m in range(2):
                    pq = ps_misc.tile([128, QB], F32, tag="mp", name=f"pq{n}{m}")
                    for c in range(8):
                        nc.tensor.matmul(
                            pq[:], wq_sb[:, c, m * 128:(m + 1) * 128],
                            xq_t[:, c, :], start=(c == 0), stop=(c == 7))
                    nc.vector.tensor_copy(qt_sb[:, m, ns], pq[:])

            def emit_vblock(n):
                ns = slice(n * QB, (n + 1) * QB)
                xv_t = xin.tile([128, 8, QB], BF16, tag="xin", name=f"xv_{n}")
                nc.scalar.dma_start(xv_t[:], xvt[n])
                for kci in range(4):
                    kc = n * 4 + kci
                    kcs = slice(kci * 128, (kci + 1) * 128)
                    pv = ps_misc.tile([128, INNER], F32, tag="mp",
                                      padded_shape=[128, 512], name=f"pv{kc}")
                    for c in range(8):
                        nc.tensor.matmul(
                            pv[:], xv_t[:, c, kcs],
                            wv_sb[:, c, :], start=(c == 0), stop=(c == 7))
                    nc.vector.tensor_copy(
                        v_sb[:, kc, :, 0:DIM_HEAD],
                        pv[:].rearrange("p (h d) -> p h d", h=HEADS_PER_CORE))

            def emit_outproj_chunk(qb, idx, tail=False):
                qc = qb * 4 + idx // 2
                dc = idx % 2
                cs = slice(qc * 128, (qc + 1) * 128)
                op = ps_misc.tile([128, 512], F32, tag="mp", name=f"op{qc}{dc}")
                for ic in range(2):
                    nc.tensor.matmul(
                        op[:], ot_sb[:, ic, cs],
                        wo_sb[:, ic, dc * 512:(dc + 1) * 512],
                        start=(ic == 0), stop=(ic == 1))
                o_stage = stage.tile([128, 512], BF16, tag="ostage",
                                     name=f"ost{qc}{dc}", bufs=3)
                nc.vector.tensor_copy(o_stage[:], op[:])
                # tail chunks alternate queues (ACT is idle by then); steady
                # state keeps pushes off the ACT engine, which paces phases
                q = nc.scalar if (tail and idx % 2 == 1) else nc.sync
                q.dma_start(out[cs, dc * 512:(dc + 1) * 512], o_stage[:])

            qb_state = {}

            def emit_st(qb, p, kc):
                qs = slice(qb * QB, (qb + 1) * QB)
                ks = slice(kc * 128, (kc + 1) * 128)
                st = ps_st.tile([128, 1024], F32, tag="st", name=f"st{qb}{p}{kc}")
                mm0 = nc.tensor.matmul(st[:, 0:512], kt_sb[0:64, p, ks],
                                       qt_sb[0:64, p, qs], start=True, stop=True)
                nc.tensor.matmul(st[:, 512:1024], kt_sb[64:128, p, ks],
                                 qt_sb[64:128, p, qs], start=True, stop=True)
                e_t = ering.tile([128, 2, 512], BF16, tag="e",
                                 name=f"e{qb}{p}{kc}")
                nc.scalar.activation(
                    e_t[:], st[:].rearrange("p (h n) -> p h n", h=2),
                    EXP, scale=float(SCALE))
                return e_t, mm0

            def emit_av(qb, p, kc, avs, e_t):
                for hh in range(2):
                    nc.tensor.matmul(
                        avs[hh][0:DIM_HEAD + 1, :],
                        v_sb[:, kc, 2 * p + hh, :], e_t[:, hh, :],
                        start=(kc == 0), stop=(kc == N_KC - 1))

            def evict_pair(qb, p, avs):
                den4 = qb_state[qb]["den4"]
                avsb = []
                for hh in range(2):
                    a_sb = stage.tile([DIM_HEAD + 1, 512], F32, tag="avsb",
                                      name=f"avsb{qb}_{p}_{hh}", bufs=4)
                    nc.vector.tensor_copy(a_sb[:], avs[hh][0:DIM_HEAD + 1, :])
                    k32 = 32 * (2 * p + hh)
                    nc.vector.tensor_copy(den4[k32:k32 + 1, :],
                                          a_sb[DIM_HEAD:DIM_HEAD + 1, :])
                    avsb.append(a_sb)
                return avsb

            def finalize_pair(qb, p, avsb, order_after=None):
                den4 = qb_state[qb]["den4"]
                qs = slice(qb * QB, (qb + 1) * QB)
                rec = stage.tile([128, 512], F32, tag="rec",
                                 name=f"rec{qb}{p}", bufs=2)
                nc.vector.reciprocal_approx_fast(rec[:], den4[:])
                recr = stage.tile([128, 512], F32R, tag="recr",
                                  name=f"recr{qb}{p}", bufs=2)
                nc.vector.tensor_copy(recr[:], rec[:])
                bc = ps_misc.tile([128, 512], F32, tag="mp", name=f"bc{qb}{p}")
                bcmm = nc.tensor.matmul(bc[:], pat_sb[:, p, :], recr[:],
                                        start=True, stop=True)
                if order_after is not None:
                    add_dep_helper(order_after.ins, bcmm.ins, sync=False,
                                   reason="hold bc behind ST stream")
                for hh in range(2):
                    nc.vector.tensor_mul(
                        ot_sb[hh * 64:(hh + 1) * 64, p, qs],
                        avsb[hh][0:DIM_HEAD, :],
                        bc[hh * 64:(hh + 1) * 64, :])


            def begin_qb(qb):
                den4 = stage.tile([128, 512], F32, tag="den4", name=f"den{qb}",
                                  bufs=1)
                nc.vector.memset(den4[:], 1.0)
                qb_state[qb] = dict(den4=den4)

            def new_avs(qb, p):
                return [ps_av.tile([128, 512], F32, tag=f"av{hh}",
                                   name=f"av{hh}_{qb}_{p}")
                        for hh in range(2)]

            def phase_fillers(qb, p):
                f = []
                if qb == 0 and p == 0:
                    for n in range(1, N_QB):
                        f.append((4 * n - 1, lambda n=n: (emit_kt(n),
                                                          emit_vblock(n))))
                elif qb == 0 and p == 1:
                    f.append((7, lambda: emit_qt(1)))
                else:
                    prev = qb - 1
                    if p == 0:
                        for g in range(4):
                            f.append(((9, 11, 13, 15)[g],
                                      lambda g=g: emit_outproj_chunk(prev, g)))
                    else:
                        if qb < N_QB - 1:
                            f.append((5, lambda: emit_qt(qb + 1)))
                        for g in range(4):
                            f.append(((3, 7, 10, 13)[g],
                                      lambda g=g: emit_outproj_chunk(prev, 4 + g)))
                return dict(f)

            emit_kt(0)
            emit_qt(0)

            wv_sb = wpool.tile([128, 8, INNER], BF16)
            nc.sync.dma_start(wv_sb[:], wv)
            # ones column of v (softmax denominator trick) — memset instead
            # of a DMA: the [128,16,4,1] scatter pattern costs 8K descriptors
            nc.vector.memset(v_sb[:, :, :, DIM_HEAD:DIM_HEAD + 1], 1.0)
            wo_sb = wpool.tile([128, 2, D_MODEL], BF16)
            nc.sync.dma_start(wo_sb[:], wo)
            pat_sb = wpool.tile([128, 2, 128], F32R)
            nc.sync.dma_start(pat_sb[:], pat4[:])

            emit_vblock(0)

            AV_LAG = 5
            phases = [(qb, p) for qb in range(N_QB) for p in range(2)]
            pending = None      # (qb, p, avs, [(kc, e_t)...])
            pending_fin = None  # (qb, p, avsb)

            for qb, p in phases:
                if p == 0:
                    begin_qb(qb)
                avs = new_avs(qb, p)
                fillers = phase_fillers(qb, p)
                eq = []
                for kc in range(N_KC):
                    e_t, stmm = emit_st(qb, p, kc)
                    eq.append((kc, e_t))
                    if kc == AV_LAG - 1 and pending is not None:
                        pq, pp, pavs, peq = pending
                        for pkc, pe_t in peq:
                            emit_av(pq, pp, pkc, pavs, pe_t)
                        pending_fin = (pq, pp, evict_pair(pq, pp, pavs))
                        pending = None
                    if kc == 6 and pending_fin is not None:
                        fq, fp, favsb = pending_fin
                        finalize_pair(fq, fp, favsb, order_after=stmm)
                        pending_fin = None
                    if kc >= AV_LAG:
                        pkc, pe_t = eq[kc - AV_LAG]
                        emit_av(qb, p, pkc, avs, pe_t)
                    if kc in fillers:
                        fillers[kc]()
                pending = (qb, p, avs, eq[N_KC - AV_LAG:])

            pq, pp, pavs, peq = pending
            for pkc, pe_t in peq:
                emit_av(pq, pp, pkc, pavs, pe_t)
            finalize_pair(pq, pp, evict_pair(pq, pp, pavs))
            for idx in range(8):
                emit_outproj_chunk(N_QB - 1, idx, tail=True)
    nc.compile()
    return nc


_NC_CACHE = None


def _get_nc():
    global _NC_CACHE
    if _NC_CACHE is None:
        _NC_CACHE = build_nc()
    return _NC_CACHE


def _make_pat4():
    pat = np.zeros((128, 2, 128), np.float32)
    for p in range(2):
        for hh in range(2):
            pat[32 * (2 * p + hh), p, hh * 64:(hh + 1) * 64] = 1.0
    return pat


def _bf16(x):
    return np.ascontiguousarray(
        np.asarray(x, np.float32).astype(ml_dtypes.bfloat16))


def _tile_x(xb):
    """[N, D] -> [N_QB, 128, 8, QB]: xT[(c p), n] viewed p-major, block-major
    so each block's DMA is contiguous per partition."""
    xt = np.asarray(xb, np.float32).T            # [D, N] = [(c p), n]
    xt = xt.reshape(8, 128, N_QB, QB)            # c, p, nb, n
    return _bf16(xt.transpose(2, 1, 0, 3))       # nb, p, c, n


def _tile_w(w):
    """[D, M] -> [128, 8, M] p-major."""
    return _bf16(np.asarray(w, np.float32).reshape(8, 128, -1).transpose(1, 0, 2))


def make_in_maps(query, key, value, Wq, Wk, Wv, Wo):
    pat4 = _make_pat4()
    in_maps = []
    for c in range(N_CORES):
        b = c // 4
        hg = c % 4
        cols = slice(hg * INNER, (hg + 1) * INNER)
        in_maps.append({
            "xqt": _tile_x(np.asarray(query[b], np.float32)),
            "xkt": _tile_x(np.asarray(key[b], np.float32)),
            "xvt": _tile_x(np.asarray(value[b], np.float32)),
            "wq": _tile_w(np.asarray(Wq)[:, cols]),
            "wk": _tile_w(np.asarray(Wk)[:, cols]),
            "wv": _tile_w(np.asarray(Wv)[:, cols]),
            "wo": _bf16(np.asarray(Wo)[cols, :].reshape(2, 128, D_MODEL)
                        .transpose(1, 0, 2)),
            "pat4": pat4,
        })
    return in_maps


def kernel(query, key, value, Wq, Wk, Wv, Wo, bo, _trace=False, _trace_cores=None):
    nc = _get_nc()
    in_maps = make_in_maps(query, key, value, Wq, Wk, Wv, Wo)
    res = bass_utils.run_bass_kernel_spmd(
        nc, in_maps, core_ids=list(range(N_CORES)), trace=_trace,
        trace_cores=_trace_cores)
    out = np.zeros((B, N, D_MODEL), np.float32)
    for c in range(N_CORES):
        out[c // 4] += np.asarray(res.results[c]["out"], np.float32)
    out += np.asarray(bo, np.float32)[None, None, :]
    if _trace:
        return out, res
    return out


# revision 3
# speedup vs baseline: 1.0059x; 1.0059x over previous
"""Multi-head attention (B=2, N=2048, d_model=1024, 16 heads x 64) on 8
Trainium2 NeuronCores — bf16 matmul variant.

Sharding: batch x head-group. Core c handles batch b = c//4 and heads
4*(c%4) .. 4*(c%4)+3. Projection weights are column-sliced (rows for Wo) so
each core computes q/k/v projections only for its 4 heads, full attention
for those heads, and a partial output projection. The host sums the four
partial outputs per batch (tensor-parallel reduce on to_out) and adds bo.

All matmul operands are bf16 (validated 2.3e-3 rel err on CPU sim vs the
2e-2 gate); accumulation stays fp32 in PSUM. Inputs/weights are converted
to bf16 on the host, halving input DMA; the partial output is stored bf16
too. The softmax normalization path (reciprocal + broadcast matmul) stays
f32/f32r for accuracy.
"""

import numpy as np
import ml_dtypes

import concourse.mybir as mybir
import concourse.tile as tile
from concourse import bacc
from concourse import bass_utils
from concourse.tile_rust import add_dep_helper

F32 = mybir.dt.float32
F32R = mybir.dt.float32r
BF16 = mybir.dt.bfloat16
EXP = mybir.ActivationFunctionType.Exp

B = 2
N = 2048
D_MODEL = 1024
NHEAD = 16
DIM_HEAD = 64
SCALE = DIM_HEAD ** (-0.5)
N_CORES = 8
HEADS_PER_CORE = 4          # 2 pairs
INNER = HEADS_PER_CORE * DIM_HEAD  # 256

QB = 512                    # query block
N_QB = N // QB              # 4
N_KC = N // 128             # 16 key chunks


def build_nc():
    nc = bacc.Bacc("TRN2", target_bir_lowering=False, debug=False,
                   num_devices=N_CORES)
    # x inputs pre-tiled on host to [block, partition, chunk, col] so each
    # block DMA is one contiguous 8KB run per partition (cheap descriptors)
    xqt = nc.dram_tensor("xqt", [N_QB, 128, 8, QB], BF16,
                         kind="ExternalInput").ap()
    xkt = nc.dram_tensor("xkt", [N_QB, 128, 8, QB], BF16,
                         kind="ExternalInput").ap()
    xvt = nc.dram_tensor("xvt", [N_QB, 128, 8, QB], BF16,
                         kind="ExternalInput").ap()
    wq = nc.dram_tensor("wq", [128, 8, INNER], BF16, kind="ExternalInput").ap()
    wk = nc.dram_tensor("wk", [128, 8, INNER], BF16, kind="ExternalInput").ap()
    wv = nc.dram_tensor("wv", [128, 8, INNER], BF16, kind="ExternalInput").ap()
    wo = nc.dram_tensor("wo", [128, 2, D_MODEL], BF16,
                        kind="ExternalInput").ap()
    # bc pattern: pat4[k, p, m] = 1 where head k owns output rows m in pair p
    pat4 = nc.dram_tensor("pat4", [128, 2, 128], F32R, kind="ExternalInput").ap()
    out = nc.dram_tensor("out", [N, D_MODEL], BF16, kind="ExternalOutput").ap()

    with tile.TileContext(nc) as tc:
        with (
            tc.tile_pool(name="wpool", bufs=1) as wpool,
            tc.tile_pool(name="persist", bufs=1) as persist,
            tc.tile_pool(name="xin", bufs=3) as xin,
            tc.tile_pool(name="ering", bufs=12) as ering,
            tc.tile_pool(name="stage", bufs=3) as stage,
            tc.tile_pool(name="ps_st", bufs=2, space="PSUM") as ps_st,
            tc.tile_pool(name="ps_av", bufs=1, space="PSUM") as ps_av,
            tc.tile_pool(name="ps_misc", bufs=2, space="PSUM") as ps_misc,
        ):
            # ---- weights on sync queue, ordered by first use; halves so
            # the first projection matmuls start on the first 4 chunks ----
            wk_sb = wpool.tile([128, 8, INNER], BF16)
            nc.sync.dma_start(wk_sb[:, 0:4, :], wk[:, 0:4, :])
            nc.sync.dma_start(wk_sb[:, 4:8, :], wk[:, 4:8, :])
            wq_sb = wpool.tile([128, 8, INNER], BF16)
            nc.sync.dma_start(wq_sb[:, 0:4, :], wq[:, 0:4, :])
            nc.sync.dma_start(wq_sb[:, 4:8, :], wq[:, 4:8, :])

            qt_sb = persist.tile([128, 2, N], BF16)
            kt_sb = persist.tile([128, 2, N], BF16)
            v_sb = persist.tile([128, N_KC, HEADS_PER_CORE, DIM_HEAD + 1], BF16)
            ot_sb = persist.tile([128, 2, N], BF16)

            def emit_kt(n):
                ns = slice(n * QB, (n + 1) * QB)
                xk_t = xin.tile([128, 8, QB], BF16, tag="xin", name=f"xk_{n}")
                nc.scalar.dma_start(xk_t[:], xkt[n])
                for m in range(2):
                    pk = ps_misc.tile([128, QB], F32, tag="mp", name=f"pk{n}{m}")
                    for c in range(8):
                        nc.tensor.matmul(
                            pk[:], wk_sb[:, c, m * 128:(m + 1) * 128],
                            xk_t[:, c, :], start=(c == 0), stop=(c == 7))
                    nc.vector.tensor_copy(kt_sb[:, m, ns], pk[:])

            def emit_qt(n):
                ns = slice(n * QB, (n + 1) * QB)
                xq_t = xin.tile([128, 8, QB], BF16, tag="xin", name=f"xq_{n}")
                nc.sync.dma_start(xq_t[:], xqt[n])
                for m in range(2):
                    pq = ps_misc.tile([128, QB], F32, tag="mp", name=f"pq{n}{m}")
                    for c in range(8):
                        nc.tensor.matmul(
                            pq[:], wq_sb[:, c, m * 128:(m + 1) * 128],
                            xq_t[:, c, :], start=(c == 0), stop=(c == 7))
                    nc.vector.tensor_copy(qt_sb[:, m, ns], pq[:])

            def emit_vblock(n):
                ns = slice(n * QB, (n + 1) * QB)
                xv_t = xin.tile([128, 8, QB], BF16, tag="xin", name=f"xv_{n}")
                nc.scalar.dma_start(xv_t[:], xvt[n])
                for kci in range(4):
                    kc = n * 4 + kci
                    kcs = slice(kci * 128, (kci + 1) * 128)
                    pv = ps_misc.tile([128, INNER], F32, tag="mp",
                                      padded_shape=[128, 512], name=f"pv{kc}")
                    for c in range(8):
                        nc.tensor.matmul(
                            pv[:], xv_t[:, c, kcs],
                            wv_sb[:, c, :], start=(c == 0), stop=(c == 7))
                    nc.vector.tensor_copy(
                        v_sb[:, kc, :, 0:DIM_HEAD],
                        pv[:].rearrange("p (h d) -> p h d", h=HEADS_PER_CORE))

            def emit_outproj_chunk(qb, idx, tail=False):
                qc = qb * 4 + idx // 2
                dc = idx % 2
                cs = slice(qc * 128, (qc + 1) * 128)
                op = ps_misc.tile([128, 512], F32, tag="mp", name=f"op{qc}{dc}")
                for ic in range(2):
                    nc.tensor.matmul(
                        op[:], ot_sb[:, ic, cs],
                        wo_sb[:, ic, dc * 512:(dc + 1) * 512],
                        start=(ic == 0), stop=(ic == 1))
                o_stage = stage.tile([128, 512], BF16, tag="ostage",
                                     name=f"ost{qc}{dc}", bufs=3)
                # tail: ACT is idle, so alternate evictions+pushes onto it,
                # halving the serial DVE latency of the last chunk pipeline
                if tail and idx % 2 == 1:
                    nc.scalar.copy(o_stage[:], op[:])
                    nc.scalar.dma_start(out[cs, dc * 512:(dc + 1) * 512],
                                        o_stage[:])
                else:
                    nc.vector.tensor_copy(o_stage[:], op[:])
                    nc.sync.dma_start(out[cs, dc * 512:(dc + 1) * 512],
                                      o_stage[:])

            qb_state = {}

            def emit_st(qb, p, kc):
                qs = slice(qb * QB, (qb + 1) * QB)
                ks = slice(kc * 128, (kc + 1) * 128)
                st = ps_st.tile([128, 1024], F32, tag="st", name=f"st{qb}{p}{kc}")
                mm0 = nc.tensor.matmul(st[:, 0:512], kt_sb[0:64, p, ks],
                                       qt_sb[0:64, p, qs], start=True, stop=True)
                nc.tensor.matmul(st[:, 512:1024], kt_sb[64:128, p, ks],
                                 qt_sb[64:128, p, qs], start=True, stop=True)
                e_t = ering.tile([128, 2, 512], BF16, tag="e",
                                 name=f"e{qb}{p}{kc}")
                nc.scalar.activation(
                    e_t[:], st[:].rearrange("p (h n) -> p h n", h=2),
                    EXP, scale=float(SCALE))
                return e_t, mm0

            def emit_av(qb, p, kc, avs, e_t):
                for hh in range(2):
                    nc.tensor.matmul(
                        avs[hh][0:DIM_HEAD + 1, :],
                        v_sb[:, kc, 2 * p + hh, :], e_t[:, hh, :],
                        start=(kc == 0), stop=(kc == N_KC - 1))

            def evict_pair(qb, p, avs):
                den4 = qb_state[qb]["den4"]
                avsb = []
                for hh in range(2):
                    a_sb = stage.tile([DIM_HEAD + 1, 512], F32, tag="avsb",
                                      name=f"avsb{qb}_{p}_{hh}", bufs=4)
                    nc.vector.tensor_copy(a_sb[:], avs[hh][0:DIM_HEAD + 1, :])
                    k32 = 32 * (2 * p + hh)
                    nc.vector.tensor_copy(den4[k32:k32 + 1, :],
                                          a_sb[DIM_HEAD:DIM_HEAD + 1, :])
                    avsb.append(a_sb)
                return avsb

            def finalize_pair(qb, p, avsb, order_after=None):
                den4 = qb_state[qb]["den4"]
                qs = slice(qb * QB, (qb + 1) * QB)
                rec = stage.tile([128, 512], F32, tag="rec",
                                 name=f"rec{qb}{p}", bufs=2)
                nc.vector.reciprocal_approx_fast(rec[:], den4[:])
                recr = stage.tile([128, 512], F32R, tag="recr",
                                  name=f"recr{qb}{p}", bufs=2)
                nc.vector.tensor_copy(recr[:], rec[:])
                bc = ps_misc.tile([128, 512], F32, tag="mp", name=f"bc{qb}{p}")
                bcmm = nc.tensor.matmul(bc[:], pat_sb[:, p, :], recr[:],
                                        start=True, stop=True)
                if order_after is not None:
                    add_dep_helper(order_after.ins, bcmm.ins, sync=False,
                                   reason="hold bc behind ST stream")
                for hh in range(2):
                    nc.vector.tensor_mul(
                        ot_sb[hh * 64:(hh + 1) * 64, p, qs],
                        avsb[hh][0:DIM_HEAD, :],
                        bc[hh * 64:(hh + 1) * 64, :])


            def begin_qb(qb):
                den4 = stage.tile([128, 512], F32, tag="den4", name=f"den{qb}",
                                  bufs=1)
                nc.vector.memset(den4[:], 1.0)
                qb_state[qb] = dict(den4=den4)

            def new_avs(qb, p):
                return [ps_av.tile([128, 512], F32, tag=f"av{hh}",
                                   name=f"av{hh}_{qb}_{p}")
                        for hh in range(2)]

            def phase_fillers(qb, p):
                f = []
                if qb == 0 and p == 0:
                    for n in range(1, N_QB):
                        f.append((4 * n - 1, lambda n=n: (emit_kt(n),
                                                          emit_vblock(n))))
                elif qb == 0 and p == 1:
                    f.append((7, lambda: emit_qt(1)))
                else:
                    prev = qb - 1
                    if p == 0:
                        for g in range(4):
                            f.append(((9, 11, 13, 15)[g],
                                      lambda g=g: emit_outproj_chunk(prev, g)))
                    else:
                        if qb < N_QB - 1:
                            f.append((5, lambda: emit_qt(qb + 1)))
                        for g in range(4):
                            f.append(((3, 7, 10, 13)[g],
                                      lambda g=g: emit_outproj_chunk(prev, 4 + g)))
                return dict(f)

            emit_kt(0)
            emit_qt(0)

            wv_sb = wpool.tile([128, 8, INNER], BF16)
            nc.sync.dma_start(wv_sb[:], wv)
            # ones column of v (softmax denominator trick) — memset instead
            # of a DMA: the [128,16,4,1] scatter pattern costs 8K descriptors
            nc.vector.memset(v_sb[:, :, :, DIM_HEAD:DIM_HEAD + 1], 1.0)
            wo_sb = wpool.tile([128, 2, D_MODEL], BF16)
            nc.sync.dma_start(wo_sb[:], wo)
            pat_sb = wpool.tile([128, 2, 128], F32R)
            nc.sync.dma_start(pat_sb[:], pat4[:])

            emit_vblock(0)

            AV_LAG = 5
            phases = [(qb, p) for qb in range(N_QB) for p in range(2)]
            pending = None      # (qb, p, avs, [(kc, e_t)...])
            pending_fin = None  # (qb, p, avsb)

            for qb, p in phases:
                if p == 0:
                    begin_qb(qb)
                avs = new_avs(qb, p)
                fillers = phase_fillers(qb, p)
                eq = []
                for kc in range(N_KC):
                    e_t, stmm = emit_st(qb, p, kc)
                    eq.append((kc, e_t))
                    if kc == AV_LAG - 1 and pending is not None:
                        pq, pp, pavs, peq = pending
                        for pkc, pe_t in peq:
                            emit_av(pq, pp, pkc, pavs, pe_t)
                        pending_fin = (pq, pp, evict_pair(pq, pp, pavs))
                        pending = None
                    if kc == 6 and pending_fin is not None:
                        fq, fp, favsb = pending_fin
                        finalize_pair(fq, fp, favsb, order_after=stmm)
                        pending_fin = None
                    if kc >= AV_LAG:
                        pkc, pe_t = eq[kc - AV_LAG]
                        emit_av(qb, p, pkc, avs, pe_t)
                    if kc in fillers:
                        fillers[kc]()
                pending = (qb, p, avs, eq[N_KC - AV_LAG:])

            pq, pp, pavs, peq = pending
            for pkc, pe_t in peq:
                emit_av(pq, pp, pkc, pavs, pe_t)
            finalize_pair(pq, pp, evict_pair(pq, pp, pavs))
            for idx in range(8):
                emit_outproj_chunk(N_QB - 1, idx, tail=True)
    nc.compile()
    return nc


_NC_CACHE = None


def _get_nc():
    global _NC_CACHE
    if _NC_CACHE is None:
        _NC_CACHE = build_nc()
    return _NC_CACHE


def _make_pat4():
    pat = np.zeros((128, 2, 128), np.float32)
    for p in range(2):
        for hh in range(2):
            pat[32 * (2 * p + hh), p, hh * 64:(hh + 1) * 64] = 1.0
    return pat


def _bf16(x):
    return np.ascontiguousarray(
        np.asarray(x, np.float32).astype(ml_dtypes.bfloat16))


def _tile_x(xb):
    """[N, D] -> [N_QB, 128, 8, QB]: xT[(c p), n] viewed p-major, block-major
    so each block's DMA is contiguous per partition."""
    xt = np.asarray(xb, np.float32).T            # [D, N] = [(c p), n]
    xt = xt.reshape(8, 128, N_QB, QB)            # c, p, nb, n
    return _bf16(xt.transpose(2, 1, 0, 3))       # nb, p, c, n


def _tile_w(w):
    """[D, M] -> [128, 8, M] p-major."""
    return _bf16(np.asarray(w, np.float32).reshape(8, 128, -1).transpose(1, 0, 2))


def make_in_maps(query, key, value, Wq, Wk, Wv, Wo):
    pat4 = _make_pat4()
    in_maps = []
    for c in range(N_CORES):
        b = c // 4
        hg = c % 4
        cols = slice(hg * INNER, (hg + 1) * INNER)
        in_maps.append({
            "xqt": _tile_x(np.asarray(query[b], np.float32)),
            "xkt": _tile_x(np.asarray(key[b], np.float32)),
            "xvt": _tile_x(np.asarray(value[b], np.float32)),
            "wq": _tile_w(np.asarray(Wq)[:, cols]),
            "wk": _tile_w(np.asarray(Wk)[:, cols]),
            "wv": _tile_w(np.asarray(Wv)[:, cols]),
            "wo": _bf16(np.asarray(Wo)[cols, :].reshape(2, 128, D_MODEL)
                        .transpose(1, 0, 2)),
            "pat4": pat4,
        })
    return in_maps


def kernel(query, key, value, Wq, Wk, Wv, Wo, bo, _trace=False, _trace_cores=None):
    nc = _get_nc()
    in_maps = make_in_maps(query, key, value, Wq, Wk, Wv, Wo)
    res = bass_utils.run_bass_kernel_spmd(
        nc, in_maps, core_ids=list(range(N_CORES)), trace=_trace,
        trace_cores=_trace_cores)
    out = np.zeros((B, N, D_MODEL), np.float32)
    for c in range(N_CORES):
        out[c // 4] += np.asarray(res.results[c]["out"], np.float32)
    out += np.asarray(bo, np.float32)[None, None, :]
    if _trace:
        return out, res
    return out
